# revision 61
# baseline (speedup 1.0000x reference)
"""Causal single-head attention on 8 TRN2 NeuronCores.

Problem: x:(S=4096, B=4, E=5) f32; Wk/Wq/Wv:(5,64), bk/bq/bv:(64,).
  K/Q/V = x@W + b per batch; scores = K.Q^T/8 (keys i, queries j), causal
  (key i attends query j iff i <= j), softmax over keys per query, out =
  sum_i V[i]*P[i,j] -> (S, B, 64).

Key algebra: scores = X6 @ M @ X6^T with X6 = [x | 1] (S,6) and
M = Wk6 @ Wq6^T / 8 (6x6).  The host precomputes Y = M @ X6^T (6,S), so
the device mm1 contracts over just 6 dims: st[key, q] = sum_c X6[key, c]
* Y[c, q] with lhsT = xt6k slice (6,128) and rhs = y6q slice (6, 512).
No on-device K/Q projections at all.

Sharding: 8 cores = 4 batches x 2 query-stripe sets (parity 0 -> query
tiles {0,1024,2048,3072}, parity 1 -> {512,1536,2560,3584}).  One SPMD
graph, static per-slot key-block profile fcnt=(4,12,20,28); per-core
differences are input data only (slack key blocks have zeroed V-side x6,
so they contribute nothing to numerator or denominator).

Device computes only XP[c, q] = sum_k x6[k, c] * exp(s_kq), accumulated
in three 6-row PSUM stripes (partitions 0:6 / 32:38 / 64:70).  Host
epilogue sums the stripes, applies Wv6 and normalizes:
out = (Wv6^T @ XP)[:64] / ones row of XP (the softmax denominator).

PE usage: each pair of key blocks runs mm1 as two concurrent matmuls in
disjoint array row-bands (stationary at partitions 0:6 / 32:38 - two
MMs writing the same PSUM bank hang the device, so lane B always lands
at PSUM col 512, its own bank).  mm2 consumes the block stream in
3-lane triples (column groups 0/32/64 - base partition 96 is illegal),
partition-disjoint within one xp bank.  Software-pipelined so mm1 of
pair i+1 issues before the mm2 units of pair i: the scalar-engine exp
stream paces the steady state with zero internal gaps.  Diagonal pairs
are ragged (block d only covers queries [128d, 512)) and interleaved
mid-slot.  Input DMAs must all land before the exp stream starts:
mid-steady-state DMA slows every engine 15-20%.
"""

import sys
from contextlib import ExitStack

import ml_dtypes
import numpy as np

for _p in ("/opt/trn_rl_repo", "/opt/pypackages"):
    if _p not in sys.path:
        sys.path.append(_p)

import concourse.bass as bass
import concourse.tile as tile
from concourse import bacc, mybir

F32 = mybir.dt.float32
BF16 = mybir.dt.bfloat16

S, B, E, NE = 4096, 4, 5, 64
N_CORES = 8
JT = 512          # query tile width
NSLOT = 4         # query tiles per core
FCNT = (4, 12, 20, 28)          # static full-unit (key-block) count per slot
NFULL = sum(FCNT)               # 64 blocks
NDIAG = NSLOT * 4               # 16 blocks
NBLK = NFULL + NDIAG            # 80 blocks in x6v
# x6v block layout: slot-descending, fulls then diags within each slot
VF_OFF = {3: 0, 2: 32, 1: 56, 0: 72}
JOS_BY_PARITY = ((0, 1024, 2048, 3072), (512, 1536, 2560, 3584))

_NC_CACHE = {}


def build_graph():
    nc = bacc.Bacc("TRN2", target_bir_lowering=False, debug=False)

    # kside = x6^T (keys); y6q/xt6q use slot-descending query-column layout
    # (processing order is slot 3,2,1,0).  The first mm1's operands ship as
    # [38,*] heads covering BOTH partition stripes in one transfer each, so
    # compute starts ~2us earlier; tails follow per stripe.
    ksh38 = nc.declare_dram_parameter("ksh38", [38, 1024], BF16, isOutput=False)
    kst = nc.declare_dram_parameter("kst", [6, S - 1024], BF16, isOutput=False)
    yqh38 = nc.declare_dram_parameter("yqh38", [38, JT], BF16, isOutput=False)
    yqt = nc.declare_dram_parameter("yqt", [6, (NSLOT - 1) * JT], BF16,
                                    isOutput=False)
    xt6q = nc.declare_dram_parameter("xt6q", [6, NSLOT * JT], BF16, isOutput=False)
    x6v = nc.declare_dram_parameter("x6v", [128, NBLK * 6], BF16, isOutput=False)
    xp48 = nc.declare_dram_parameter("xp48", [NSLOT * 70, JT], F32, isOutput=True)

    with tile.TileContext(nc) as tc, ExitStack() as ctx:
        consts = ctx.enter_context(tc.tile_pool(name="consts", bufs=1))
        psum = ctx.enter_context(tc.tile_pool(name="psum", bufs=2, space="PSUM"))
        sb = ctx.enter_context(tc.tile_pool(name="sb", bufs=2))

        # inputs, two partition-stripe copies (rows 0:6 / 32:38), issued on
        # both HWDGE rings (sync + scalar) ordered by first use
        ks_sb = consts.tile([38, S], BF16)
        yq_sb = consts.tile([38, NSLOT * JT], BF16)
        xq_sb = consts.tile([38, NSLOT * JT], BF16)
        x6v_sb = consts.tile([128, NBLK, 6], BF16)
        # Heads first (one DMA covers both stripes), then x6v (needed by the
        # first mm2), then the tails; everything lands by the time the exp
        # stream ramps (mid-steady-state DMA slows every engine ~15-20%)
        nc.sync.dma_start(out=ks_sb[0:38, 0:1024], in_=ksh38[:])
        nc.scalar.dma_start(out=yq_sb[0:38, 0:JT], in_=yqh38[:])
        nc.scalar.dma_start(
            out=x6v_sb[:], in_=x6v[:].rearrange("p (n c) -> p n c", c=6)
        )
        nc.sync.dma_start(out=ks_sb[0:6, 1024:], in_=kst[:])
        nc.scalar.dma_start(out=yq_sb[0:6, JT:], in_=yqt[:])
        nc.sync.dma_start(out=ks_sb[32:38, 1024:], in_=kst[:])
        nc.scalar.dma_start(out=yq_sb[32:38, JT:], in_=yqt[:])
        nc.sync.dma_start(out=xq_sb[0:6, :], in_=xt6q[:])
        nc.sync.dma_start(out=xq_sb[32:38, :], in_=xt6q[:])

        def kq(r0, c0, w):  # stripe r0 (0 or 32), cols [c0, c0+w)
            if c0 < S:
                return ks_sb[r0 : r0 + 6, c0 : c0 + w]
            if c0 < S + NSLOT * JT:
                c0 -= S
                return xq_sb[r0 : r0 + 6, c0 : c0 + w]

            c0 -= S + NSLOT * JT
            return yq_sb[r0 : r0 + 6, c0 : c0 + w]

        XT6Q0, Y6Q0 = S, S + NSLOT * JT

        # Ragged diag masks.  Diag block d covers only queries [128d, 512), so
        # each block needs a [128,128] lower-triangle mask (p <= q) on its
        # leading 128 query columns, ones elsewhere.  Lane B always lands at
        # PSUM col 512 (its own bank — two concurrent matmuls must not write
        # the same bank), so dp=1 has a dont-care gap at [256:512) (zeroed).
        #   dp=0 seg [0:896):    [tri 0:128|ones ..512|tri 512:640|ones ..896]
        #   dp=1 seg [896:1536): [tri 896:1024|ones ..1152|zero ..1408|tri ..1536]
        masks_f = consts.tile([128, 1536], F32)
        nc.gpsimd.memset(masks_f[:], 1.0)
        nc.gpsimd.memset(masks_f[:, 1152:1408], 0.0)
        for c0 in (0, 512, 896, 1408):
            nc.gpsimd.affine_select(
                out=masks_f[:, c0 : c0 + 128],
                in_=masks_f[:, c0 : c0 + 128],
                compare_op=mybir.AluOpType.is_ge,
                fill=0.0,
                base=0,
                pattern=[[1, 128]],
                channel_multiplier=-1,
            )
        masks_sb = consts.tile([128, 1536], BF16)
        nc.vector.tensor_copy(masks_sb[:], masks_f[:])

        # ---- build the global pair list; each pair = two concurrent lanes
        # lane = (stripe, lhs_col, rhs_off, width, st_off, xp_off); diag pairs
        # are ragged (block d only covers queries [128d, 512)) and are
        # interleaved mid-slot ----
        pairs = []
        slots = []  # (t, [global pair indices])
        for t in (3, 2, 1, 0):
            nf = FCNT[t]
            scol = (3 - t) * JT  # slot-descending query-column base
            voff = VF_OFF[t]
            slot_pairs = []
            for up in range(nf // 2):
                ue, uo = 2 * up, 2 * up + 1
                slot_pairs.append(
                    dict(
                        t=t,
                        laneA=(0, ue * 128, 0, JT, 0, 0),
                        laneB=(32, uo * 128, 0, JT, JT, 0),
                        xvA=voff + ue, xvB=voff + uo,
                        mask=None,
                    )
                )
            for dp in range(2):
                de, do = 2 * dp, 2 * dp + 1
                wa, wb = JT - 128 * de, JT - 128 * do
                dpair = dict(
                    t=t,
                    laneA=(0, XT6Q0 + scol + de * 128, de * 128, wa, 0,
                           de * 128),
                    laneB=(32, XT6Q0 + scol + do * 128, do * 128, wb, JT,
                           do * 128),
                    xvA=voff + nf + de, xvB=voff + nf + do,
                    mask=(0, 896) if dp == 0 else (896, 640),
                )
                # spread the two diag pairs through the slot
                slot_pairs.insert((nf // 6 + 1) * (dp + 1), dpair)
            base = len(pairs)
            pairs.extend(slot_pairs)
            slots.append((t, list(range(base, base + len(slot_pairs)))))

        NP = len(pairs)  # 40
        pt_tiles = [None] * NP
        xp_by_slot = {}

        # ---- mm2 units: each slot's block stream grouped into 3-lane
        # triples (PSUM stripes 0/32/64, partition-disjoint in one bank);
        # leftovers run as pairs.  lane = (stripe, pair, half-st_off, w,
        # xp_off, xv) ----
        mm2_units = []
        for t, pidx in slots:
            blocks = []
            for i in pidx:
                p = pairs[i]
                for lane, xv in ((p["laneA"], p["xvA"]), (p["laneB"], p["xvB"])):
                    _, _, _, w, st_off, xp_off = lane
                    blocks.append((i, st_off, w, xp_off, xv))
            slot_units = []
            k = 0
            while k < len(blocks):
                n = min(3, len(blocks) - k)
                if len(blocks) - k == 4:
                    n = 2  # split a trailing 4 as 2+2
                lanes = [(32 * j,) + blocks[k + j] for j in range(n)]
                slot_units.append(
                    dict(t=t, lanes=lanes,
                         max_pair=max(b[0] for b in blocks[k : k + n]))
                )
                k += n
            slot_units[0]["start"] = True
            last_by_stripe = {}
            for ui, un in enumerate(slot_units):
                for lane in un["lanes"]:
                    last_by_stripe[lane[0]] = ui
            for ui, un in enumerate(slot_units):
                un["stops"] = {
                    s for s, last in last_by_stripe.items() if last == ui
                }
            slot_units[-1]["flush"] = True
            mm2_units.extend(slot_units)

        def emit_mm1(i):
            p = pairs[i]
            jcol = Y6Q0 + (3 - p["t"]) * JT
            st = psum.tile([128, 2 * JT], F32, tag="st", bufs=3)
            pt = sb.tile([128, 2 * JT], BF16, tag="pt", bufs=6)
            pt_tiles[i] = pt
            wtot = 0
            for r, lhs_col, rhs_off, w, st_off, _ in (p["laneA"], p["laneB"]):
                nc.tensor.matmul(
                    st[:, st_off : st_off + w],
                    kq(r, lhs_col, 128),
                    kq(r, jcol + rhs_off, w),
                    start=True, stop=True,
                )
                wtot = st_off + w
            nc.scalar.activation(
                pt[:, 0:wtot], st[:, 0:wtot], mybir.ActivationFunctionType.Exp
            )
            if p["mask"] is not None:
                seg, w = p["mask"]
                nc.vector.tensor_mul(
                    pt[:, 0:w], pt[:, 0:w], masks_sb[:, seg : seg + w]
                )

        def emit_mm2_unit(q):
            un = mm2_units[q]
            t = un["t"]
            if un.get("start"):
                xp_by_slot[t] = psum.tile(
                    [70, JT], F32, tag="xp", bufs=2, name=f"xp{t}"
                )
            xp = xp_by_slot[t]
            start = un.get("start", False)
            for r, pi, st_off, w, xp_off, xv in un["lanes"]:
                nc.tensor.matmul(
                    xp[r : r + 6, xp_off : xp_off + w],
                    x6v_sb[:, xv, :],
                    pt_tiles[pi][:, st_off : st_off + w],
                    start=start, stop=(r in un["stops"]),
                    skip_group_check=True,
                )
            if un.get("flush"):
                # one copy covers all three stripes
                xps = sb.tile([70, JT], F32, tag="xps", bufs=2)
                nc.vector.tensor_copy(xps[:], xp[:])
                nc.sync.dma_start(
                    out=xp48[t * 70 : (t + 1) * 70, :], in_=xps[:]
                )

        # software pipeline: mm1 runs one pair ahead of the mm2 units
        emit_mm1(0)
        uq = 0
        for i in range(1, NP):
            emit_mm1(i)
            while uq < len(mm2_units) and mm2_units[uq]["max_pair"] <= i - 1:
                emit_mm2_unit(uq)
                uq += 1
        while uq < len(mm2_units):
            emit_mm2_unit(uq)
            uq += 1

    nc.compile()
    return nc


def make_in_maps(x, Wk, bk, Wq, bq, Wv, bv):
    """Build the 8 per-core input dicts from the full problem inputs."""
    x = np.asarray(x, np.float64)
    wk6 = np.vstack([np.asarray(Wk, np.float64), np.asarray(bk, np.float64)[None]])
    wq6 = np.vstack([np.asarray(Wq, np.float64), np.asarray(bq, np.float64)[None]])
    m66 = (wk6 @ wq6.T) / 8.0  # (6, 6): scores = X6 @ m66 @ X6^T

    in_maps = []
    for core in range(N_CORES):
        b, parity = core // 2, core % 2
        jos = JOS_BY_PARITY[parity]
        x6 = np.concatenate([x[:, b, :], np.ones((S, 1), np.float64)], axis=1)
        y6 = m66 @ x6.T  # (6, S)

        # slot-descending query-column layouts (slot 3 first)
        tdesc = (3, 2, 1, 0)
        xt6q = np.concatenate([x6[jos[t] : jos[t] + JT].T for t in tdesc], axis=1)
        y6q = np.concatenate([y6[:, jos[t] : jos[t] + JT] for t in tdesc], axis=1)

        x6v = np.zeros((128, NBLK, 6), np.float64)
        for t, jo in enumerate(jos):
            v0 = VF_OFF[t]
            blk = x6[: FCNT[t] * 128].copy().reshape(FCNT[t], 128, 6)
            blk[jo // 128 :] = 0.0  # slack blocks: V-side zeroed
            x6v[:, v0 : v0 + FCNT[t], :] = blk.transpose(1, 0, 2)
            dblk = x6[jo : jo + JT].reshape(4, 128, 6)
            x6v[:, v0 + FCNT[t] : v0 + FCNT[t] + 4, :] = dblk.transpose(1, 0, 2)

        ks = np.ascontiguousarray(x6.T).astype(ml_dtypes.bfloat16)
        yqb = np.ascontiguousarray(y6q).astype(ml_dtypes.bfloat16)
        ksh38 = np.zeros((38, 1024), ml_dtypes.bfloat16)
        ksh38[0:6] = ks[:, 0:1024]
        ksh38[32:38] = ks[:, 0:1024]
        yqh38 = np.zeros((38, JT), ml_dtypes.bfloat16)
        yqh38[0:6] = yqb[:, 0:JT]
        yqh38[32:38] = yqb[:, 0:JT]
        in_maps.append(
            {
                "ksh38": ksh38,
                "kst": ks[:, 1024:].copy(),
                "yqh38": yqh38,
                "yqt": yqb[:, JT:].copy(),
                "xt6q": np.ascontiguousarray(xt6q).astype(ml_dtypes.bfloat16),
                "x6v": np.ascontiguousarray(x6v.reshape(128, NBLK * 6)).astype(
                    ml_dtypes.bfloat16
                ),
            }
        )
    return in_maps


def assemble_output(results, Wv, bv):
    """Host epilogue: apply Wv6, normalize, stitch into (S, B, NE)."""
    wv6 = np.vstack([np.asarray(Wv, np.float64), np.asarray(bv, np.float64)[None]])
    out = np.zeros((S, B, NE), np.float32)
    for core in range(N_CORES):
        b, parity = core // 2, core % 2
        jos = JOS_BY_PARITY[parity]
        xp48 = np.asarray(results[core]["xp48"], np.float64)  # (280, 512)
        for t, jo in enumerate(jos):
            sl = xp48[t * 70 : (t + 1) * 70]
            xp = sl[0:6] + sl[32:38] + sl[64:70]
            num = wv6.T @ xp  # (64, 512): Wv^T x-moments + bv * ones-row
            out[jo : jo + JT, b, :] = (num / xp[5]).T
    return out


def run_on_device(in_maps, trace=False):
    from concourse.bass_utils import run_bass_kernel_spmd

    if "nc" not in _NC_CACHE:
        _NC_CACHE["nc"] = build_graph()
    nc = _NC_CACHE["nc"]
    return run_bass_kernel_spmd(
        nc, in_maps, core_ids=list(range(N_CORES)), trace=trace
    )


def kernel(x, Wk, bk, Wq, bq, Wv, bv):
    in_maps = make_in_maps(x, Wk, bk, Wq, bq, Wv, bv)
    res = run_on_device(in_maps, trace=False)
    return assemble_output(res.results, Wv, bv)
